# revision 4
# baseline (speedup 1.0000x reference)
"""BGAT layer (batched graph attention) on 8 Trainium2 NeuronCores.

Data-parallel over batch: each core processes B/8 = 8 batches.
Per batch b (N=1024 nodes, C=F=512):
  h = x[b] @ W                           [N, F]
  s1 = x[b] @ (W @ a1), s2 = x[b] @ (W @ a2)   (associativity: (xW)a == x(Wa))
  eT[j, i] = leaky_relu(s1[i] + s2[j]) * maskT[j, i]    (transposed layout)
  pT = exp(eT)  (softmax w/o max-subtraction: e in [-2, ~8], safe in fp32)
  denom[i] = sum_j pT[j, i]  (ones-lhsT matmul)
  u[i, f] = sum_j pT[j, i] * h[j, f]  (pT slices are the matmul lhsT directly)
  out = elu(u / denom + beta * h)
The transposed score layout makes softmax numerator tiles feed the second
matmul as stationary operands with no on-chip transposes at all.
"""

import sys
from contextlib import ExitStack

import numpy as np

for _p in ("/opt/trn_rl_repo", "/opt/pypackages"):
    if _p not in sys.path:
        sys.path.append(_p)

import ml_dtypes  # noqa: E402
import concourse.tile as tile  # noqa: E402
from concourse import mybir, bacc  # noqa: E402
import concourse.bass_utils as bass_utils  # noqa: E402

B, N, C, F = 64, 1024, 512, 512
NCORES = 8
BPC = B // NCORES  # batches per core
CT = C // 128      # contraction tiles
NT = N // 128      # node tiles
ALPHA = 0.2

F32 = mybir.dt.float32
F32R = mybir.dt.float32r
BF16 = mybir.dt.bfloat16
ALU = mybir.AluOpType
ACT = mybir.ActivationFunctionType

_programs = {}


def _build(beta: float):
    nc = bacc.Bacc("TRN2", debug=False)

    xT_d = nc.dram_tensor("xT", [BPC, C, N], F32R, kind="ExternalInput").ap()
    W_d = nc.dram_tensor("W", [C, F], F32R, kind="ExternalInput").ap()
    wa_d = nc.dram_tensor("wa", [C, 2], F32R, kind="ExternalInput").ap()
    maskT_d = nc.dram_tensor("maskT", [N, N], BF16, kind="ExternalInput").ap()
    ones_d = nc.dram_tensor("ones", [128, 1], F32R, kind="ExternalInput").ap()
    out_d = nc.dram_tensor("out", [BPC, N, F], F32, kind="ExternalOutput").ap()

    with tile.TileContext(nc) as tc, ExitStack() as es:
        const = es.enter_context(tc.tile_pool(name="const", bufs=1))
        xpool = es.enter_context(tc.tile_pool(name="xT", bufs=2))
        hpool = es.enter_context(tc.tile_pool(name="h", bufs=2))
        ppool = es.enter_context(tc.tile_pool(name="p", bufs=2))
        spool = es.enter_context(tc.tile_pool(name="s", bufs=2))
        lpool = es.enter_context(tc.tile_pool(name="l", bufs=3))
        opool = es.enter_context(tc.tile_pool(name="o", bufs=3))
        qpool = es.enter_context(tc.tile_pool(name="q", bufs=3))
        rpool = es.enter_context(tc.tile_pool(name="r", bufs=2))
        dstp = es.enter_context(tc.tile_pool(name="dst", bufs=2, space="DRAM"))
        dd = es.enter_context(tc.tile_pool(name="dd", bufs=2, space="DRAM"))
        ps_h = es.enter_context(tc.tile_pool(name="ps_h", bufs=2, space="PSUM"))
        ps_s = es.enter_context(tc.tile_pool(name="ps_s", bufs=1, space="PSUM"))
        ps_u = es.enter_context(tc.tile_pool(name="ps_u", bufs=2, space="PSUM"))
        ps_d = es.enter_context(tc.tile_pool(name="ps_d", bufs=1, space="PSUM"))

        # constants
        W_t = const.tile([128, CT, F], F32R)
        for ct in range(CT):
            nc.sync.dma_start(out=W_t[:, ct, :], in_=W_d[ct * 128:(ct + 1) * 128, :])
        wa_t = const.tile([128, CT, 2], F32R)
        for ct in range(CT):
            nc.sync.dma_start(out=wa_t[:, ct, :], in_=wa_d[ct * 128:(ct + 1) * 128, :])
        mask_t = const.tile([128, NT, N], BF16)
        for jt in range(NT):
            nc.sync.dma_start(out=mask_t[:, jt, :], in_=maskT_d[jt * 128:(jt + 1) * 128, :])
        ones_c = const.tile([128, 1], F32R)
        nc.sync.dma_start(out=ones_c, in_=ones_d)

        def emit_mm2(b, p_t, h_t):
            # denominators: denom[i] = sum_j pT[j, i] -> psum row [1, N]
            pden = ps_d.tile([1, N], F32)
            for jt in range(NT):
                for hf in range(2):
                    nc.tensor.matmul(
                        pden[:, hf * 512:(hf + 1) * 512],
                        lhsT=ones_c,
                        rhs=p_t[:, jt, hf * 512:(hf + 1) * 512],
                        start=(jt == 0), stop=(jt == NT - 1),
                    )
            den_sb = rpool.tile([1, N], F32)
            nc.vector.tensor_copy(out=den_sb, in_=pden)
            d_t = dd.tile([1, N], F32)
            nc.sync.dma_start(out=d_t, in_=den_sb)
            rd = rpool.tile([128, NT], F32)
            nc.sync.dma_start(out=rd, in_=d_t.rearrange("one (i p) -> one p i", p=128).squeeze(0))
            nc.vector.reciprocal(out=rd, in_=rd)

            for it in range(NT):
                pu = ps_u.tile([128, F], F32)
                for jt in range(NT):
                    nc.tensor.matmul(
                        pu,
                        lhsT=p_t[:, jt, it * 128:(it + 1) * 128],
                        rhs=h_t[:, jt, :],
                        start=(jt == 0), stop=(jt == NT - 1),
                    )
                o_t = opool.tile([128, F], F32)
                h_it = h_t[:, it, :].bitcast(F32)
                if beta == 1.0:
                    # o = pu * (1/denom) + h
                    nc.vector.scalar_tensor_tensor(
                        out=o_t, in0=pu, scalar=rd[:, it:it + 1], in1=h_it,
                        op0=ALU.mult, op1=ALU.add)
                else:
                    nc.vector.tensor_scalar_mul(o_t, pu, rd[:, it:it + 1])
                    nc.vector.scalar_tensor_tensor(
                        out=o_t, in0=h_it, scalar=float(beta), in1=o_t,
                        op0=ALU.mult, op1=ALU.add)
                # elu(o) = max(o, min(exp(o), 1) - 1)
                q_t = qpool.tile([128, F], F32)
                nc.scalar.activation(out=q_t, in_=o_t, func=ACT.Exp)
                nc.vector.tensor_scalar(out=q_t, in0=q_t, scalar1=1.0, scalar2=-1.0,
                                        op0=ALU.min, op1=ALU.add)
                nc.vector.tensor_max(o_t, o_t, q_t)
                nc.sync.dma_start(out=out_d[b, it * 128:(it + 1) * 128, :], in_=o_t)

        prev = None
        for b in range(BPC):
            xT_t = xpool.tile([128, CT, N], F32R)
            for ct in range(CT):
                nc.sync.dma_start(out=xT_t[:, ct, :], in_=xT_d[b, ct * 128:(ct + 1) * 128, :])

            # mm1: h tiles [nt] = sum_ct xT[ct, nt].T @ W[ct]
            h_t = hpool.tile([128, NT, F], F32R)
            for nt in range(NT):
                ph = ps_h.tile([128, F], F32)
                for ct in range(CT):
                    nc.tensor.matmul(
                        ph,
                        lhsT=xT_t[:, ct, nt * 128:(nt + 1) * 128],
                        rhs=W_t[:, ct, :],
                        start=(ct == 0), stop=(ct == CT - 1),
                    )
                nc.scalar.copy(out=h_t[:, nt, :], in_=ph)

            # s rows: [2, N] = wa.T @ xT
            pst = ps_s.tile([2, 2, 512], F32)
            for ct in range(CT):
                for hf in range(2):
                    nc.tensor.matmul(
                        pst[:, hf, :],
                        lhsT=wa_t[:, ct, :],
                        rhs=xT_t[:, ct, hf * 512:(hf + 1) * 512],
                        start=(ct == 0), stop=(ct == CT - 1),
                    )
            st_sb = spool.tile([2, 2, 512], F32)
            nc.vector.tensor_copy(out=st_sb, in_=pst)
            st_t = dstp.tile([2, N], F32)
            nc.sync.dma_start(out=st_t.rearrange("r (h c) -> r h c", h=2), in_=st_sb)
            s1b = spool.tile([128, N], F32)
            nc.sync.dma_start(out=s1b, in_=st_t[0:1, :].to_broadcast((128, N)))
            s2c = spool.tile([128, NT], F32)
            nc.sync.dma_start(out=s2c, in_=st_t[1:2, :].rearrange("one (j p) -> one p j", p=128).squeeze(0))

            # e-stage: pT[j, i] = exp(leaky(s1[i] + s2[j]) * maskT[j, i])
            p_t = ppool.tile([128, NT, N], F32R)
            for jt in range(NT):
                l_t = lpool.tile([128, N], F32)
                nc.scalar.activation(out=l_t, in_=s1b, func=ACT.Prelu,
                                     bias=s2c[:, jt:jt + 1], scale=1.0, alpha=ALPHA)
                nc.vector.tensor_tensor(out=l_t, in0=l_t, in1=mask_t[:, jt, :], op=ALU.mult)
                nc.scalar.activation(out=p_t[:, jt, :], in_=l_t, func=ACT.Exp)

            if prev is not None:
                emit_mm2(*prev)
            prev = (b, p_t, h_t)
        emit_mm2(*prev)

    nc.compile()
    return nc


def make_in_maps(x, W, a, mask):
    xT = np.ascontiguousarray(x.transpose(0, 2, 1))                  # [B, C, N]
    maskT = np.ascontiguousarray(mask.T).astype(ml_dtypes.bfloat16)  # exact: mask is 0/1
    wa = np.concatenate([W @ a[:F, 0:1], W @ a[F:, 0:1]], axis=1).astype(np.float32)
    ones = np.ones((128, 1), dtype=np.float32)
    return [
        {"xT": xT[i * BPC:(i + 1) * BPC], "W": W, "wa": wa, "maskT": maskT, "ones": ones}
        for i in range(NCORES)
    ]


def kernel(x, W, a, beta, mask):
    x = np.asarray(x, dtype=np.float32)
    W = np.asarray(W, dtype=np.float32)
    a = np.asarray(a, dtype=np.float32)
    mask = np.asarray(mask, dtype=np.float32)
    beta_val = float(np.asarray(beta).reshape(-1)[0])

    key = beta_val
    if key not in _programs:
        _programs[key] = _build(beta_val)
    nc = _programs[key]

    in_maps = make_in_maps(x, W, a, mask)
    res = bass_utils.run_bass_kernel_spmd(nc, in_maps, core_ids=list(range(NCORES)))
    return np.concatenate([res.results[i]["out"] for i in range(NCORES)], axis=0)
